# revision 3
# baseline (speedup 1.0000x reference)
"""CrossNet (DCN cross layers) forward on 8 Trainium2 NeuronCores.

Math: xl_{t+1} = x0 * (xl_t . w_t) + b_t + xl_t  stays in the affine span of
x0: xl_t = a_t * x0 + c_t with c_t = sum_{j<t} b_j a constant vector and a_t a
per-row scalar.  With u_t = x0 . w_t and g_t = c_t . w_t (weight-only consts):

    a_{t+1} = a_t * (1 + u_t) + g_t ,  a_0 = 1
    out     = a_L * x0 + sum_t b_t

So the whole network is 3 independent per-row dot products (u0,u1,u2), a tiny
scalar recurrence, and one scale-and-add -- one read of x, one write of out.

Sharding: data-parallel on batch, 2048 rows per core, weights replicated.

Engine split per [128,1024] row tile, mode "balanced" (DMA roofline is
~2.9us/tile/core; every engine is kept under it):

  DVE    u0,u1 via native scalar_tensor_tensor w/ accum_out (fp32 1x mode,
         which never contends with GPSIMD for the shared SBUF port), plus the
         tiny a3 recurrence (1x ops only)
  GPSIMD scr = x * w2 (the third dot-product multiply)
  ACT    accum-reduce of scr -> u2, diag(a3) build, PSUM->SBUF copy
  PE     out_psum = diag(a3) @ x0  (fp32, 4 cyc/col)
                  + ones2 @ [dsum_hi; dsum_lo]  (bf16 K=2 rank-2 broadcast --
         dsum split into two bf16 parts keeps fp32-level accuracy at 1 cyc/col)
  DMA    HWDGE loads/stores, 512KB each

Mode "vector" keeps all three dots on DVE (no GPSIMD) -- simpler, ~15% slower.
"""

import os

import numpy as np

import concourse.mybir as mybir
from concourse.bacc import Bacc
from concourse.bass import Bass
from concourse.bass_utils import run_bass_kernel_spmd
from concourse.masks import make_identity
from concourse.tile import TileContext

B, D, L = 16384, 1024, 3
N_CORES = 8
RPC = B // N_CORES  # rows per core: 2048
P = 128
N_TILES = RPC // P  # 16
F32 = mybir.dt.float32
BF16 = mybir.dt.bfloat16

MODE = os.environ.get("CROSSNET_MODE", "balanced")
# Benchmark-only: repeat the whole body N times inside one program so
# per-iteration HW time can be extracted as a slope (axon has no NTFF).
REPEAT = int(os.environ.get("CROSSNET_REPEAT", "1"))
# Benchmark-only: trace=True to pull an NTFF profile back through axon.
TRACE = os.environ.get("CROSSNET_TRACE", "0") == "1"
LAST_RESULTS = None

_CACHE: dict[str, Bass] = {}


def _build() -> Bass:
    # Bacc (not plain Bass): its compile() runs move_matmul_waits_to_ldweights
    # + generate_event_semaphores, which walrus needs -- matmul/LDW can carry
    # at most one sync wait.
    nc = Bacc("TRN2", target_bir_lowering=False, debug=False, num_devices=N_CORES)
    x = nc.dram_tensor("x", [RPC, D], F32, kind="ExternalInput")
    # aux rows: 0..2 = w0,w1,w2 ; 3 = dsum (unused in balanced) ; 4 = [g1, g2]
    aux = nc.dram_tensor("aux", [5, D], F32, kind="ExternalInput")
    # dsum split into bf16 hi/lo rows: dsum ~= hi + lo to ~2^-16 relative.
    aux16 = nc.dram_tensor("aux16", [2, D], BF16, kind="ExternalInput")
    out = nc.dram_tensor("out", [RPC, D], F32, kind="ExternalOutput")

    mult = mybir.AluOpType.mult
    add = mybir.AluOpType.add
    Copy = mybir.ActivationFunctionType.Copy
    Ident = mybir.ActivationFunctionType.Identity

    with TileContext(nc) as tc:
        with (
            tc.tile_pool(name="consts", bufs=1) as consts,
            tc.tile_pool(name="xp", bufs=6) as xp,
            tc.tile_pool(name="scrp", bufs=3) as scrp,
            tc.tile_pool(name="op", bufs=4) as op,
            tc.tile_pool(name="small", bufs=8) as small,
            tc.tile_pool(name="diagp", bufs=3) as diagp,
            tc.tile_pool(name="psum", bufs=2, space="PSUM") as psum_pool,
            tc.tile_pool(name="psum_bc", bufs=2, space="PSUM") as psum_bc,
        ):
            # ---- one-time constants ----
            # Each aux row lands in its own [1, D] tile at partition 0 so it
            # can be a matmul rhs alongside lhsT tiles based at partition 0.
            w_row = []
            for t in range(3):
                r = consts.tile([1, D], F32, tag=f"w_row{t}")
                nc.sync.dma_start(out=r, in_=aux[t : t + 1, :])
                w_row.append(r)
            g_row = consts.tile([1, 2], F32, tag="g_row")
            nc.sync.dma_start(out=g_row, in_=aux[4:5, 0:2])
            d16 = consts.tile([2, D], BF16, tag="d16")
            nc.sync.dma_start(out=d16, in_=aux16[:, :])

            ones_col = consts.tile([1, P], F32, tag="ones_col")
            nc.vector.memset(ones_col, 1.0)
            ones2_bf = consts.tile([2, P], BF16, tag="ones2_bf")
            nc.vector.memset(ones2_bf, 1.0)
            ones4 = consts.tile([P, 4], F32, tag="ones4")
            nc.vector.memset(ones4, 1.0)
            ident = consts.tile([P, P], F32, tag="ident")
            make_identity(nc, ident)

            # Broadcast w_t and [g1,g2] across partitions via PE outer
            # product: ones[1,P]^T @ row[1,N] -> [P, N] in PSUM.
            wb = []
            for t in range(3):
                wt = consts.tile([P, D], F32, tag=f"wb{t}")
                for h in range(2):
                    sl = slice(512 * h, 512 * (h + 1))
                    ps = psum_bc.tile([P, 512], F32, tag="bc")
                    nc.tensor.matmul(ps, ones_col, w_row[t][:, sl], start=True, stop=True)
                    nc.scalar.copy(wt[:, sl], ps)
                wb.append(wt)
            gb = consts.tile([P, 2], F32, tag="gb")
            ps = psum_bc.tile([P, 2], F32, tag="bc_g")
            nc.tensor.matmul(ps, ones_col, g_row[:, 0:2], start=True, stop=True)
            nc.scalar.copy(gb, ps)

            # ---- steady-state row tiles ----
            for i in range(N_TILES * REPEAT):
                i = i % N_TILES
                rows = slice(i * P, (i + 1) * P)
                xt = xp.tile([P, D], F32, tag="x")
                nc.sync.dma_start(out=xt, in_=x[rows, :])

                u = small.tile([P, 4], F32, tag="u")
                scr = small.tile([P, D], F32, tag="scr")

                if MODE == "balanced":
                    # third dot product: multiply on GPSIMD, reduce on ACT
                    scr2 = scrp.tile([P, D], F32, tag="scr2")
                    nc.gpsimd.tensor_tensor(scr2, xt, wb[2], op=mult)
                    dummy2 = small.tile([P, 1], F32, tag="dummy2")
                    nc.scalar.activation(
                        dummy2.broadcast_to((P, D)), scr2, Copy,
                        accum_out=u[:, 2:3],
                    )
                for t in range(2):
                    nc.vector.scalar_tensor_tensor(
                        out=scr, in0=xt, scalar=1.0, in1=wb[t],
                        op0=mult, op1=mult, accum_out=u[:, t : t + 1],
                    )
                if MODE != "balanced":
                    nc.vector.scalar_tensor_tensor(
                        out=scr, in0=xt, scalar=1.0, in1=wb[2],
                        op0=mult, op1=mult, accum_out=u[:, 2:3],
                    )

                # recurrence: v = 1+u ; a2 = v0*v1+g1 ; a3 = a2*v2+g2
                v = small.tile([P, 3], F32, tag="v")
                a2 = small.tile([P, 1], F32, tag="a2")
                a3 = small.tile([P, 1], F32, tag="a3")
                if MODE == "balanced":
                    # tiny 1x DVE ops (tensor_scalar would be a 2-port op and
                    # block behind long GPSIMD instructions)
                    nc.vector.tensor_tensor(v, u[:, 0:3], ones4[:, 0:3], op=add)
                    nc.vector.scalar_tensor_tensor(
                        out=a2, in0=v[:, 0:1], scalar=v[:, 1:2], in1=gb[:, 0:1],
                        op0=mult, op1=add,
                    )
                    nc.vector.scalar_tensor_tensor(
                        out=a3, in0=a2, scalar=v[:, 2:3], in1=gb[:, 1:2],
                        op0=mult, op1=add,
                    )
                else:
                    nc.scalar.activation(v, u[:, 0:3], Ident, bias=1.0)
                    nc.scalar.activation(
                        a2, v[:, 0:1], Ident, scale=v[:, 1:2], bias=gb[:, 0:1]
                    )
                    nc.scalar.activation(
                        a3, a2, Ident, scale=v[:, 2:3], bias=gb[:, 1:2]
                    )
                diag = diagp.tile([P, P], F32, tag="diag")
                nc.scalar.activation(diag, ident, Copy, scale=a3[:, 0:1])

                # out_psum = diag(a3) @ x0 + ones2 @ [dsum_hi; dsum_lo]
                # (diag loaded once, then the bf16 K=2 stationary once)
                ps_out = psum_pool.tile([P, D], F32, tag="ps_out")
                for h in range(2):
                    sl = slice(512 * h, 512 * (h + 1))
                    nc.tensor.matmul(
                        ps_out[:, sl], diag, xt[:, sl], start=True, stop=False
                    )
                for h in range(2):
                    sl = slice(512 * h, 512 * (h + 1))
                    nc.tensor.matmul(
                        ps_out[:, sl], ones2_bf, d16[:, sl], start=False, stop=True
                    )

                ot = op.tile([P, D], F32, tag="ot")
                nc.scalar.copy(ot, ps_out)
                nc.sync.dma_start(out=out[rows, :], in_=ot)

    nc.compile()
    return nc


def _get_program() -> Bass:
    key = f"{MODE}-{REPEAT}"
    if key not in _CACHE:
        _CACHE[key] = _build()
    return _CACHE[key]


def _make_aux(weights: np.ndarray, bias: np.ndarray):
    import ml_dtypes

    w = np.asarray(weights, dtype=np.float32)
    b = np.asarray(bias, dtype=np.float32)
    aux = np.zeros((5, D), dtype=np.float32)
    aux[0:3] = w
    dsum = b.sum(axis=0)
    aux[3] = dsum
    aux[4, 0] = float(b[0] @ w[1])
    aux[4, 1] = float((b[0] + b[1]) @ w[2])
    hi = dsum.astype(ml_dtypes.bfloat16)
    lo = (dsum - hi.astype(np.float32)).astype(ml_dtypes.bfloat16)
    aux16 = np.stack([hi, lo])
    return aux, aux16


def kernel(x: np.ndarray, weights: np.ndarray, bias: np.ndarray) -> np.ndarray:
    x = np.ascontiguousarray(np.asarray(x, dtype=np.float32))
    aux, aux16 = _make_aux(weights, bias)
    nc = _get_program()
    in_maps = [
        {"x": x[i * RPC : (i + 1) * RPC], "aux": aux, "aux16": aux16}
        for i in range(N_CORES)
    ]
    res = run_bass_kernel_spmd(nc, in_maps, list(range(N_CORES)), trace=TRACE)
    global LAST_RESULTS
    LAST_RESULTS = res
    return np.concatenate([r["out"] for r in res.results], axis=0)

